# revision 9
# baseline (speedup 1.0000x reference)
"""Conv2D 3x3 (stride 1, pad 1) Bass kernel for Trainium2, 8 NeuronCores.

Problem: x (32,128,56,56) f32, Wk (256,128,3,3) f32, b (256,) f32
         -> out (32,256,56,56) f32

Strategy:
  - Data-parallel over batch: 4 images per core, 8 cores. No collectives.
  - Implicit GEMM, 9 filter taps accumulated in PSUM per output tile
    [oc_chunk(128) x 8 rows x 56 cols]. Contraction dim = in_c = 128.
  - 2 of the 9 taps ((0,0) and (1,0)) run as ONE fp8e4 DoubleRow matmul:
    the PE virtualizes to 128x256 (2 fp8 weights/cell, 2 MACs/cycle), so
    the pair streams 448 columns once instead of twice. 8 matmul slots per
    tile instead of 9 -> ~11% fewer PE column-cycles.
      * fp8 operand scales split as (8*W) x (x/8) so e4m3 stays in its
        normal range on both sides and the product needs no rescale.
      * DoubleRow k-tile stride must be 16B-aligned, so the image is
        padded to width 64: the two taps sit exactly 64 elements apart.
      * accuracy: 2 taps in e4m3 -> rel_fro ~1.64e-2 (vs 2e-2 budget);
        the other 7 taps stay fp16 (~3e-4 each).
  - x staged in SBUF zero-padded to 58x64 in BOTH dtypes (fp16 for the 7
    taps, fp8 for the pair); all taps are pure strided slices.
  - Bias (fp32) added during PSUM->SBUF evacuation on the DVE.
  - Output stores alternate sync/scalar trigger engines -> two HWDGE
    queues, so store bandwidth tracks the faster matmul cadence.
"""

import numpy as np
import ml_dtypes

import bass_rust
import concourse.bacc as bacc
import concourse.bass as bass  # noqa: F401  (engine classes referenced via nc)
import concourse.mybir as mybir
from concourse.bass_utils import run_bass_kernel_spmd
from concourse.tile import TileContext

B, IN_C, OUT_C, H, W, KS = 32, 128, 256, 56, 56, 3
N_CORES = 8
B_PER = B // N_CORES           # 4 images per core
HP, WPAD = H + 2, 64           # fp8 image: rows 58, cols padded to 64
WP16 = H + 2                   # fp16 image keeps baseline 58-wide rows
RB = 8                         # output rows per matmul block
N_RB = H // RB                 # 7 row blocks
P = 128
OC_CHUNKS = OUT_C // P         # 2

X8_DIV = 8.0                   # x quantized as e4m3(x/8), W as e4m3(8*W)

FP8_TAPS = [(0, 0), (1, 0)]    # DoubleRow pair: k-tile stride = WPAD = 64
FP16_TAPS = [(2, 0), (0, 1), (1, 1), (2, 1), (0, 2), (1, 2), (2, 2)]
ALL_TAPS = [(kh, kw) for kh in range(KS) for kw in range(KS)]

F8 = mybir.dt.float8e4
F16 = mybir.dt.float16
F32 = mybir.dt.float32


def _custom_ap(base, dims):
    """Replace the free dims of `base` with explicit (stride, count) pairs
    (elements). Partition dim is kept. Used for the DoubleRow rhs whose two
    k-tiles are overlapping 1-row-shifted windows of the same image."""
    c = base.copy()
    part = list(base.ap[0])
    c.ap = bass_rust.VecI64Pair([part] + [[s, n] for (s, n) in dims])
    return c


def _build_program():
    # Bacc (not raw Bass): its finalize() runs the wait-splitting passes
    # (each TRN2 instruction can carry at most one sync wait).
    nc = bacc.Bacc("TRN2", target_bir_lowering=False)

    x16_ext = nc.declare_dram_parameter("x16", [IN_C, B_PER, HP, WP16], F16, isOutput=False)
    x8_ext = nc.declare_dram_parameter("x8", [IN_C, B_PER, HP, WPAD], F8, isOutput=False)
    w16_ext = nc.declare_dram_parameter("w16", [IN_C, len(FP16_TAPS), OUT_C], F16, isOutput=False)
    # SW-interleaved DoubleRow weights: per oc-chunk a [IN_C, 256] block with
    # flat[:, 2*j+i] = ktile_i[:, 127-j] (see bass_interp DoubleRowSwInterleave)
    w8_ext = nc.declare_dram_parameter("w8", [IN_C, OC_CHUNKS, 2 * P], F8, isOutput=False)
    # fp16 copies of the pair taps, for the all-fp16 final pieces
    w16f_ext = nc.declare_dram_parameter("w16f", [IN_C, 2, OUT_C], F16, isOutput=False)
    b_ext = nc.declare_dram_parameter("b", [P, OC_CHUNKS], F32, isOutput=False)
    o_ext = nc.declare_dram_parameter("out", [B_PER, OUT_C, H, W], F32, isOutput=True)

    with TileContext(nc) as tc:
        with (
            tc.tile_pool(name="const", bufs=1) as cpool,
            tc.tile_pool(name="psum", bufs=7, space="PSUM") as ppool,
            tc.tile_pool(name="warmp", bufs=1, space="PSUM") as dwarm,
            tc.tile_pool(name="outp", bufs=8) as opool,
        ):
            x16_sb = cpool.tile([IN_C, B_PER, HP, WP16], F16, name="x16_sb")
            x8_sb = cpool.tile([IN_C, B_PER, HP, WPAD], F8, name="x8_sb")
            w16_sb = cpool.tile([IN_C, len(FP16_TAPS), OUT_C], F16, name="w16_sb")
            w8_sb = cpool.tile([IN_C, OC_CHUNKS, 2 * P], F8, name="w8_sb")
            w16f_sb = cpool.tile([IN_C, 2, OUT_C], F16, name="w16f_sb")
            b_sb = cpool.tile([P, OC_CHUNKS], F32, name="b_sb")

            # Staging. Trigger instructions cost ~0.6us on the issuing
            # engine and each engine owns its own HWDGE queue, so spread:
            #   scalar: weights (small, needed first) then output stores
            #   sync:   x16 (+ half the output stores later)
            #   gpsimd: x8 (otherwise idle engine)
            # First chunks are small so the first tile's inputs land ASAP.
            # scalar: weights. w16 split by oc-chunk so tile0 (ci=0) has all
            # 7 fp16 taps as soon as the second transfer lands.
            nc.scalar.dma_start(out=w8_sb[:], in_=w8_ext[:])
            nc.scalar.dma_start(out=w16_sb[:, :, 0:P], in_=w16_ext[:, :, 0:P])
            nc.scalar.dma_start(out=w16_sb[:, :, P:OUT_C], in_=w16_ext[:, :, P:OUT_C])
            nc.scalar.dma_start(out=b_sb[:], in_=b_ext[:])
            nc.scalar.dma_start(out=w16f_sb[:], in_=w16f_ext[:])
            # sync: fp16 image chunks in consumption order
            nc.sync.dma_start(out=x16_sb[:, 0, 0:10], in_=x16_ext[:, 0, 0:10])
            nc.sync.dma_start(out=x16_sb[:, 0, 10:26], in_=x16_ext[:, 0, 10:26])
            nc.sync.dma_start(out=x16_sb[:, 0, 26:42], in_=x16_ext[:, 0, 26:42])
            nc.sync.dma_start(out=x16_sb[:, 0, 42:58], in_=x16_ext[:, 0, 42:58])
            # gpsimd: fp8 image chunks (consumed 1 row-block ahead of fp16)
            nc.gpsimd.dma_start(out=x8_sb[:, 0, 0:10], in_=x8_ext[:, 0, 0:10])
            nc.gpsimd.dma_start(out=x8_sb[:, 0, 10:34], in_=x8_ext[:, 0, 10:34])
            nc.gpsimd.dma_start(out=x8_sb[:, 0, 34:58], in_=x8_ext[:, 0, 34:58])
            for n in range(1, B_PER):
                for r0, r1 in [(0, 29), (29, 58)]:
                    nc.sync.dma_start(out=x16_sb[:, n, r0:r1], in_=x16_ext[:, n, r0:r1])
                    nc.gpsimd.dma_start(out=x8_sb[:, n, r0:r1], in_=x8_ext[:, n, r0:r1])

            # Pre-warm the PE HAM clock gate while the first chunks are in
            # flight: the first real matmul can start ~1.5us in, so only a
            # short burst is needed to bridge; HAM finishes warming during
            # the first real matmuls.
            warm_sb = cpool.tile([P, 128], mybir.dt.bfloat16, name="warm_sb")
            warm_ps = dwarm.tile([P, 128], F32, name="warm_ps")
            nc.vector.memset(warm_sb[:], 0)
            for i in range(26):
                nc.tensor.matmul(
                    warm_ps[:],
                    lhsT=warm_sb[:],
                    rhs=warm_sb[:],
                    start=(i == 0),
                    stop=False,
                    skip_group_check=True,
                )

            def store_tile(o_dst, ot, nrows, store):
                if store == "split":
                    h = nrows // 2
                    nc.sync.dma_start(out=o_dst[:, 0:h], in_=ot[:, 0:h])
                    nc.scalar.dma_start(out=o_dst[:, h:nrows], in_=ot[:, h:nrows])
                elif store == "sync":
                    nc.sync.dma_start(out=o_dst, in_=ot[:])
                else:
                    nc.scalar.dma_start(out=o_dst, in_=ot[:])

            def emit_tile(n, ci, row0, nrows, store):
                # PSUM tile shaped full-RB so small tail pieces never share
                # a bank with a neighbor (start=True zero-region safety);
                # the written region [P, nrows, 56] is contiguous.
                ps = ppool.tile([P, RB, W], F32, name="ps", tag="ps")
                # fp8 DoubleRow pair: taps (0,0)+(1,0) in one matmul.
                # rhs free dims: [k-tile(2) stride 64][rows stride 64][56]
                rhs = _custom_ap(
                    x8_sb[:, n, row0 : row0 + nrows + 1],
                    [(WPAD, 2), (WPAD, nrows), (1, W)],
                )
                nc.tensor.matmul(
                    ps[:, 0:nrows],
                    lhsT=w8_sb[:, ci],
                    rhs=rhs,
                    start=True,
                    stop=False,
                    perf_mode=mybir.MatmulPerfMode.DoubleRowSwInterleave,
                )
                for j, (kh, kw) in enumerate(FP16_TAPS):
                    nc.tensor.matmul(
                        ps[:, 0:nrows],
                        lhsT=w16_sb[:, j, ci * P : (ci + 1) * P],
                        rhs=x16_sb[:, n, row0 + kh : row0 + kh + nrows, kw : kw + W],
                        start=False,
                        stop=(j == len(FP16_TAPS) - 1),
                    )
                ot = opool.tile([P, nrows, W], F32, name="ot", tag="ot")
                # explicit DVE: nc.any routes this to ScalarE, which is
                # ~9x slower for plain copy+add and becomes the bottleneck
                nc.vector.tensor_scalar_add(ot[:], ps[:, 0:nrows], b_sb[:, ci : ci + 1])
                o_dst = o_ext[n, ci * P : (ci + 1) * P, row0 : row0 + nrows, :]
                store_tile(o_dst, ot, nrows, store)

            def emit_tile_fp16(n, ci, row0, nrows, store):
                # all-fp16 tile (used for the shrinking tail pieces)
                ps = ppool.tile([P, RB, W], F32, name="ps", tag="ps")
                for t, (kh, kw) in enumerate(ALL_TAPS):
                    if (kh, kw) in FP8_TAPS:
                        lhsT = w16f_sb[:, FP8_TAPS.index((kh, kw)), ci * P : (ci + 1) * P]
                    else:
                        lhsT = w16_sb[:, FP16_TAPS.index((kh, kw)), ci * P : (ci + 1) * P]
                    nc.tensor.matmul(
                        ps[:, 0:nrows],
                        lhsT=lhsT,
                        rhs=x16_sb[:, n, row0 + kh : row0 + kh + nrows, kw : kw + W],
                        start=(t == 0),
                        stop=(t == KS * KS - 1),
                    )
                ot = opool.tile([P, nrows, W], F32, name="ot", tag="ot")
                nc.vector.tensor_scalar_add(ot[:], ps[:, 0:nrows], b_sb[:, ci : ci + 1])
                o_dst = o_ext[n, ci * P : (ci + 1) * P, row0 : row0 + nrows, :]
                store_tile(o_dst, ot, nrows, store)

            tile_idx = 0
            for n in range(B_PER):
                for rb in range(N_RB):
                    for ci in range(OC_CHUNKS):
                        last_rb = n == B_PER - 1 and rb == N_RB - 1
                        if last_rb and ci == OC_CHUNKS - 1:
                            # final tile as three shrinking all-fp16 pieces on
                            # alternating store engines so evacuation + store
                            # overlap the tail instead of serializing
                            emit_tile_fp16(n, ci, rb * RB + 0, 3, "scalar")
                            emit_tile_fp16(n, ci, rb * RB + 3, 3, "sync")
                            emit_tile_fp16(n, ci, rb * RB + 6, 2, "scalar")
                        elif tile_idx >= 48:
                            # tail tiles: halve per-queue store chunks so the
                            # final drain doesn't serialize on one queue
                            emit_tile(n, ci, rb * RB, RB, "split")
                        else:
                            # alternate store queues so neither HWDGE queue
                            # has to sustain the full 151 GB/s output rate
                            emit_tile(n, ci, rb * RB, RB, "scalar" if tile_idx % 2 == 0 else "sync")
                        tile_idx += 1
    nc.finalize()  # Bacc.finalize runs the wait-splitting compile passes
    return nc


_NC_CACHE = {}


def _get_program():
    if "nc" not in _NC_CACHE:
        _NC_CACHE["nc"] = _build_program()
    return _NC_CACHE["nc"]


def _prep_inputs(x, Wk, b):
    x = np.asarray(x, dtype=np.float32)
    Wk = np.asarray(Wk, dtype=np.float32)
    b = np.asarray(b, dtype=np.float32)

    # weights: [oc, ic, kh, kw] -> [ic, tap, oc] slices per dtype
    wt = Wk.transpose(1, 2, 3, 0)  # [ic, kh, kw, oc]
    w16 = np.ascontiguousarray(
        np.stack([wt[:, kh, kw] for (kh, kw) in FP16_TAPS], axis=1).astype(np.float16)
    )
    w16f = np.ascontiguousarray(
        np.stack([wt[:, kh, kw] for (kh, kw) in FP8_TAPS], axis=1).astype(np.float16)
    )
    w8log = np.clip(
        np.stack([wt[:, kh, kw] for (kh, kw) in FP8_TAPS], axis=1) * X8_DIV,
        -240.0,
        240.0,
    ).astype(ml_dtypes.float8_e4m3)  # [ic, 2, oc]
    # SwInterleave layout per oc-chunk: flat[:, 2*j+i] = ktile_i[:, 127-j]
    w8 = np.zeros((IN_C, OC_CHUNKS, 2 * P), dtype=ml_dtypes.float8_e4m3)
    for ci in range(OC_CHUNKS):
        blk = w8log[:, :, ci * P : (ci + 1) * P]  # [ic, 2, 128]
        w8[:, ci, 0::2] = blk[:, 0, ::-1]
        w8[:, ci, 1::2] = blk[:, 1, ::-1]
    w8 = np.ascontiguousarray(w8)
    b_prep = np.ascontiguousarray(b.reshape(OC_CHUNKS, P).T)

    # x: [b, ic, h, w] -> per-core [ic, b_per, 58, 64] zero-padded, 2 dtypes
    x_pad16 = np.zeros((B, IN_C, HP, WP16), dtype=np.float16)
    x_pad16[:, :, 1 : H + 1, 1 : W + 1] = x
    x_pad8 = np.zeros((B, IN_C, HP, WPAD), dtype=ml_dtypes.float8_e4m3)
    x_pad8[:, :, 1 : H + 1, 1 : W + 1] = np.clip(x / X8_DIV, -240.0, 240.0).astype(
        ml_dtypes.float8_e4m3
    )

    in_maps = []
    for c in range(N_CORES):
        sl = slice(c * B_PER, (c + 1) * B_PER)
        in_maps.append(
            {
                "x16": np.ascontiguousarray(x_pad16[sl].transpose(1, 0, 2, 3)),
                "x8": np.ascontiguousarray(x_pad8[sl].transpose(1, 0, 2, 3)),
                "w16": w16,
                "w8": w8,
                "w16f": w16f,
                "b": b_prep,
            }
        )
    return in_maps


def run(x, Wk, b, **spmd_kwargs):
    """Run the conv on 8 cores; returns (full_output, BassKernelResults)."""
    nc = _get_program()
    in_maps = _prep_inputs(x, Wk, b)
    try:
        res = run_bass_kernel_spmd(nc, in_maps, list(range(N_CORES)), **spmd_kwargs)
    except Exception:
        # transient NRT device errors have been observed to recover on retry
        import time

        time.sleep(2.0)
        res = run_bass_kernel_spmd(nc, in_maps, list(range(N_CORES)), **spmd_kwargs)
    out = np.concatenate([res.results[i]["out"] for i in range(N_CORES)], axis=0)
    return out, res


def kernel(x, Wk, b):
    out, _ = run(x, Wk, b)
    return out


# revision 10
# speedup vs baseline: 1.0873x; 1.0873x over previous
"""Conv2D 3x3 (stride 1, pad 1) Bass kernel for Trainium2, 8 NeuronCores.

Problem: x (32,128,56,56) f32, Wk (256,128,3,3) f32, b (256,) f32
         -> out (32,256,56,56) f32

Strategy:
  - Data-parallel over batch: 4 images per core, 8 cores. No collectives.
  - Implicit GEMM, 9 filter taps accumulated in PSUM per output tile
    [oc_chunk(128) x 8 rows x 56 cols]. Contraction dim = in_c = 128.
  - 2 of the 9 taps ((0,0) and (1,0)) run as ONE fp8e4 DoubleRow matmul:
    the PE virtualizes to 128x256 (2 fp8 weights/cell, 2 MACs/cycle), so
    the pair streams 448 columns once instead of twice. 8 matmul slots per
    tile instead of 9 -> ~11% fewer PE column-cycles.
      * fp8 operand scales split as (8*W) x (x/8) so e4m3 stays in its
        normal range on both sides and the product needs no rescale.
      * DoubleRow k-tile stride must be 16B-aligned, so the image is
        padded to width 64: the two taps sit exactly 64 elements apart.
      * accuracy: 2 taps in e4m3 -> rel_fro ~1.64e-2 (vs 2e-2 budget);
        the other 7 taps stay fp16 (~3e-4 each).
  - x staged in SBUF zero-padded to 58x64 in BOTH dtypes (fp16 for the 7
    taps, fp8 for the pair); all taps are pure strided slices.
  - Bias (fp32) added during PSUM->SBUF evacuation on the DVE.
  - Output stores alternate sync/scalar trigger engines -> two HWDGE
    queues, so store bandwidth tracks the faster matmul cadence.
"""

import numpy as np
import ml_dtypes

import bass_rust
import concourse.bacc as bacc
import concourse.bass as bass  # noqa: F401  (engine classes referenced via nc)
import concourse.mybir as mybir
from concourse.bass_utils import run_bass_kernel_spmd
from concourse.tile import TileContext

B, IN_C, OUT_C, H, W, KS = 32, 128, 256, 56, 56, 3
N_CORES = 8
B_PER = B // N_CORES           # 4 images per core
HP, WPAD = H + 2, 64           # fp8 image: rows 58, cols padded to 64
WP16 = H + 2                   # fp16 image keeps baseline 58-wide rows
RB = 8                         # output rows per matmul block
N_RB = H // RB                 # 7 row blocks
P = 128
OC_CHUNKS = OUT_C // P         # 2

X8_DIV = 8.0                   # x quantized as e4m3(x/8), W as e4m3(8*W)

FP8_TAPS = [(0, 0), (1, 0)]    # DoubleRow pair: k-tile stride = WPAD = 64
FP16_TAPS = [(2, 0), (0, 1), (1, 1), (2, 1), (0, 2), (1, 2), (2, 2)]
ALL_TAPS = [(kh, kw) for kh in range(KS) for kw in range(KS)]

F8 = mybir.dt.float8e4
F16 = mybir.dt.float16
F32 = mybir.dt.float32


def _custom_ap(base, dims):
    """Replace the free dims of `base` with explicit (stride, count) pairs
    (elements). Partition dim is kept. Used for the DoubleRow rhs whose two
    k-tiles are overlapping 1-row-shifted windows of the same image."""
    c = base.copy()
    part = list(base.ap[0])
    c.ap = bass_rust.VecI64Pair([part] + [[s, n] for (s, n) in dims])
    return c


def _build_program():
    # Bacc (not raw Bass): its finalize() runs the wait-splitting passes
    # (each TRN2 instruction can carry at most one sync wait).
    nc = bacc.Bacc("TRN2", target_bir_lowering=False)

    x16_ext = nc.declare_dram_parameter("x16", [IN_C, B_PER, HP, WP16], F16, isOutput=False)
    x8_ext = nc.declare_dram_parameter("x8", [IN_C, B_PER, HP, WPAD], F8, isOutput=False)
    # oc-chunk-major so each chunk stages as one contiguous DMA
    w16_ext = nc.declare_dram_parameter("w16", [IN_C, OC_CHUNKS, len(FP16_TAPS), P], F16, isOutput=False)
    # SW-interleaved DoubleRow weights: per oc-chunk a [IN_C, 256] block with
    # flat[:, 2*j+i] = ktile_i[:, 127-j] (see bass_interp DoubleRowSwInterleave)
    w8_ext = nc.declare_dram_parameter("w8", [IN_C, OC_CHUNKS, 2 * P], F8, isOutput=False)
    # fp16 copies of the pair taps, for the all-fp16 final pieces
    w16f_ext = nc.declare_dram_parameter("w16f", [IN_C, 2, OUT_C], F16, isOutput=False)
    b_ext = nc.declare_dram_parameter("b", [P, OC_CHUNKS], F32, isOutput=False)
    o_ext = nc.declare_dram_parameter("out", [B_PER, OUT_C, H, W], F32, isOutput=True)

    with TileContext(nc) as tc:
        with (
            tc.tile_pool(name="const", bufs=1) as cpool,
            tc.tile_pool(name="psum", bufs=7, space="PSUM") as ppool,
            tc.tile_pool(name="warmp", bufs=1, space="PSUM") as dwarm,
            tc.tile_pool(name="outp", bufs=8) as opool,
        ):
            x16_sb = cpool.tile([IN_C, B_PER, HP, WP16], F16, name="x16_sb")
            x8_sb = cpool.tile([IN_C, B_PER, HP, WPAD], F8, name="x8_sb")
            w16_sb = cpool.tile([IN_C, OC_CHUNKS, len(FP16_TAPS), P], F16, name="w16_sb")
            w8_sb = cpool.tile([IN_C, OC_CHUNKS, 2 * P], F8, name="w8_sb")
            w16f_sb = cpool.tile([IN_C, 2, OUT_C], F16, name="w16f_sb")
            b_sb = cpool.tile([P, OC_CHUNKS], F32, name="b_sb")

            # Staging. Trigger instructions cost ~0.6us on the issuing
            # engine and each engine owns its own HWDGE queue, so spread:
            #   scalar: weights (small, needed first) then output stores
            #   sync:   x16 (+ half the output stores later)
            #   gpsimd: x8 (otherwise idle engine)
            # First chunks are small so the first tile's inputs land ASAP.
            # scalar: weights. w16 split by oc-chunk so tile0 (ci=0) has all
            # 7 fp16 taps as soon as the second transfer lands.
            nc.scalar.dma_start(out=w8_sb[:], in_=w8_ext[:])
            nc.scalar.dma_start(out=w16_sb[:, 0], in_=w16_ext[:, 0])
            nc.scalar.dma_start(out=w16_sb[:, 1], in_=w16_ext[:, 1])
            nc.scalar.dma_start(out=b_sb[:], in_=b_ext[:])
            nc.scalar.dma_start(out=w16f_sb[:], in_=w16f_ext[:])
            # sync: fp16 image chunks in consumption order
            nc.sync.dma_start(out=x16_sb[:, 0, 0:10], in_=x16_ext[:, 0, 0:10])
            nc.sync.dma_start(out=x16_sb[:, 0, 10:26], in_=x16_ext[:, 0, 10:26])
            nc.sync.dma_start(out=x16_sb[:, 0, 26:42], in_=x16_ext[:, 0, 26:42])
            nc.sync.dma_start(out=x16_sb[:, 0, 42:58], in_=x16_ext[:, 0, 42:58])
            # gpsimd: fp8 image chunks (consumed 1 row-block ahead of fp16)
            nc.gpsimd.dma_start(out=x8_sb[:, 0, 0:10], in_=x8_ext[:, 0, 0:10])
            nc.gpsimd.dma_start(out=x8_sb[:, 0, 10:34], in_=x8_ext[:, 0, 10:34])
            nc.gpsimd.dma_start(out=x8_sb[:, 0, 34:58], in_=x8_ext[:, 0, 34:58])
            for n in range(1, B_PER):
                for r0, r1 in [(0, 29), (29, 58)]:
                    nc.sync.dma_start(out=x16_sb[:, n, r0:r1], in_=x16_ext[:, n, r0:r1])
                    nc.gpsimd.dma_start(out=x8_sb[:, n, r0:r1], in_=x8_ext[:, n, r0:r1])

            # Pre-warm the PE HAM clock gate while the first chunks are in
            # flight: the first real matmul can start ~1.5us in, so only a
            # short burst is needed to bridge; HAM finishes warming during
            # the first real matmuls.
            warm_sb = cpool.tile([P, 128], mybir.dt.bfloat16, name="warm_sb")
            warm_ps = dwarm.tile([P, 128], F32, name="warm_ps")
            nc.vector.memset(warm_sb[:], 0)
            for i in range(26):
                nc.tensor.matmul(
                    warm_ps[:],
                    lhsT=warm_sb[:],
                    rhs=warm_sb[:],
                    start=(i == 0),
                    stop=False,
                    skip_group_check=True,
                )

            def store_tile(o_dst, ot, nrows, store):
                if store == "split":
                    h = nrows // 2
                    nc.sync.dma_start(out=o_dst[:, 0:h], in_=ot[:, 0:h])
                    nc.scalar.dma_start(out=o_dst[:, h:nrows], in_=ot[:, h:nrows])
                elif store == "sync":
                    nc.sync.dma_start(out=o_dst, in_=ot[:])
                else:
                    nc.scalar.dma_start(out=o_dst, in_=ot[:])

            def emit_tile(n, ci, row0, nrows, store):
                # PSUM tile shaped full-RB so small tail pieces never share
                # a bank with a neighbor (start=True zero-region safety);
                # the written region [P, nrows, 56] is contiguous.
                ps = ppool.tile([P, RB, W], F32, name="ps", tag="ps")
                # fp8 DoubleRow pair: taps (0,0)+(1,0) in one matmul.
                # rhs free dims: [k-tile(2) stride 64][rows stride 64][56]
                rhs = _custom_ap(
                    x8_sb[:, n, row0 : row0 + nrows + 1],
                    [(WPAD, 2), (WPAD, nrows), (1, W)],
                )
                nc.tensor.matmul(
                    ps[:, 0:nrows],
                    lhsT=w8_sb[:, ci],
                    rhs=rhs,
                    start=True,
                    stop=False,
                    perf_mode=mybir.MatmulPerfMode.DoubleRowSwInterleave,
                )
                for j, (kh, kw) in enumerate(FP16_TAPS):
                    nc.tensor.matmul(
                        ps[:, 0:nrows],
                        lhsT=w16_sb[:, ci, j],
                        rhs=x16_sb[:, n, row0 + kh : row0 + kh + nrows, kw : kw + W],
                        start=False,
                        stop=(j == len(FP16_TAPS) - 1),
                    )
                ot = opool.tile([P, nrows, W], F32, name="ot", tag="ot")
                # explicit DVE: nc.any routes this to ScalarE, which is
                # ~9x slower for plain copy+add and becomes the bottleneck
                nc.vector.tensor_scalar_add(ot[:], ps[:, 0:nrows], b_sb[:, ci : ci + 1])
                o_dst = o_ext[n, ci * P : (ci + 1) * P, row0 : row0 + nrows, :]
                store_tile(o_dst, ot, nrows, store)

            def emit_tile_fp16(n, ci, row0, nrows, store):
                # all-fp16 tile (used for the shrinking tail pieces)
                ps = ppool.tile([P, RB, W], F32, name="ps", tag="ps")
                for t, (kh, kw) in enumerate(ALL_TAPS):
                    if (kh, kw) in FP8_TAPS:
                        lhsT = w16f_sb[:, FP8_TAPS.index((kh, kw)), ci * P : (ci + 1) * P]
                    else:
                        lhsT = w16_sb[:, ci, FP16_TAPS.index((kh, kw))]
                    nc.tensor.matmul(
                        ps[:, 0:nrows],
                        lhsT=lhsT,
                        rhs=x16_sb[:, n, row0 + kh : row0 + kh + nrows, kw : kw + W],
                        start=(t == 0),
                        stop=(t == KS * KS - 1),
                    )
                ot = opool.tile([P, nrows, W], F32, name="ot", tag="ot")
                nc.vector.tensor_scalar_add(ot[:], ps[:, 0:nrows], b_sb[:, ci : ci + 1])
                o_dst = o_ext[n, ci * P : (ci + 1) * P, row0 : row0 + nrows, :]
                store_tile(o_dst, ot, nrows, store)

            tile_idx = 0
            for n in range(B_PER):
                for rb in range(N_RB):
                    for ci in range(OC_CHUNKS):
                        last_rb = n == B_PER - 1 and rb == N_RB - 1
                        if last_rb and ci == OC_CHUNKS - 1:
                            # final tile as three shrinking all-fp16 pieces on
                            # alternating store engines so evacuation + store
                            # overlap the tail instead of serializing
                            emit_tile_fp16(n, ci, rb * RB + 0, 3, "scalar")
                            emit_tile_fp16(n, ci, rb * RB + 3, 3, "sync")
                            emit_tile_fp16(n, ci, rb * RB + 6, 2, "scalar")
                        elif tile_idx >= 48:
                            # tail tiles: halve per-queue store chunks so the
                            # final drain doesn't serialize on one queue
                            emit_tile(n, ci, rb * RB, RB, "split")
                        else:
                            # alternate store queues so neither HWDGE queue
                            # has to sustain the full 151 GB/s output rate
                            emit_tile(n, ci, rb * RB, RB, "scalar" if tile_idx % 2 == 0 else "sync")
                        tile_idx += 1
    nc.finalize()  # Bacc.finalize runs the wait-splitting compile passes
    return nc


_NC_CACHE = {}


def _get_program():
    if "nc" not in _NC_CACHE:
        _NC_CACHE["nc"] = _build_program()
    return _NC_CACHE["nc"]


def _prep_inputs(x, Wk, b):
    x = np.asarray(x, dtype=np.float32)
    Wk = np.asarray(Wk, dtype=np.float32)
    b = np.asarray(b, dtype=np.float32)

    # weights: [oc, ic, kh, kw] -> [ic, tap, oc] slices per dtype
    wt = Wk.transpose(1, 2, 3, 0)  # [ic, kh, kw, oc]
    w16_full = np.stack([wt[:, kh, kw] for (kh, kw) in FP16_TAPS], axis=1)  # [ic,7,oc]
    w16 = np.ascontiguousarray(
        w16_full.reshape(IN_C, len(FP16_TAPS), OC_CHUNKS, P)
        .transpose(0, 2, 1, 3)
        .astype(np.float16)
    )
    w16f = np.ascontiguousarray(
        np.stack([wt[:, kh, kw] for (kh, kw) in FP8_TAPS], axis=1).astype(np.float16)
    )
    w8log = np.clip(
        np.stack([wt[:, kh, kw] for (kh, kw) in FP8_TAPS], axis=1) * X8_DIV,
        -240.0,
        240.0,
    ).astype(ml_dtypes.float8_e4m3)  # [ic, 2, oc]
    # SwInterleave layout per oc-chunk: flat[:, 2*j+i] = ktile_i[:, 127-j]
    w8 = np.zeros((IN_C, OC_CHUNKS, 2 * P), dtype=ml_dtypes.float8_e4m3)
    for ci in range(OC_CHUNKS):
        blk = w8log[:, :, ci * P : (ci + 1) * P]  # [ic, 2, 128]
        w8[:, ci, 0::2] = blk[:, 0, ::-1]
        w8[:, ci, 1::2] = blk[:, 1, ::-1]
    w8 = np.ascontiguousarray(w8)
    b_prep = np.ascontiguousarray(b.reshape(OC_CHUNKS, P).T)

    # x: [b, ic, h, w] -> per-core [ic, b_per, 58, 64] zero-padded, 2 dtypes
    x_pad16 = np.zeros((B, IN_C, HP, WP16), dtype=np.float16)
    x_pad16[:, :, 1 : H + 1, 1 : W + 1] = x
    x_pad8 = np.zeros((B, IN_C, HP, WPAD), dtype=ml_dtypes.float8_e4m3)
    x_pad8[:, :, 1 : H + 1, 1 : W + 1] = np.clip(x / X8_DIV, -240.0, 240.0).astype(
        ml_dtypes.float8_e4m3
    )

    in_maps = []
    for c in range(N_CORES):
        sl = slice(c * B_PER, (c + 1) * B_PER)
        in_maps.append(
            {
                "x16": np.ascontiguousarray(x_pad16[sl].transpose(1, 0, 2, 3)),
                "x8": np.ascontiguousarray(x_pad8[sl].transpose(1, 0, 2, 3)),
                "w16": w16,
                "w8": w8,
                "w16f": w16f,
                "b": b_prep,
            }
        )
    return in_maps


def run(x, Wk, b, **spmd_kwargs):
    """Run the conv on 8 cores; returns (full_output, BassKernelResults)."""
    nc = _get_program()
    in_maps = _prep_inputs(x, Wk, b)
    try:
        res = run_bass_kernel_spmd(nc, in_maps, list(range(N_CORES)), **spmd_kwargs)
    except Exception:
        # transient NRT device errors have been observed to recover on retry
        import time

        time.sleep(2.0)
        res = run_bass_kernel_spmd(nc, in_maps, list(range(N_CORES)), **spmd_kwargs)
    out = np.concatenate([res.results[i]["out"] for i in range(N_CORES)], axis=0)
    return out, res


def kernel(x, Wk, b):
    out, _ = run(x, Wk, b)
    return out
